# revision 4
# baseline (speedup 1.0000x reference)
"""Camera2World Trainium2 Bass kernel (bf16-IO version).

out[b,n,i,h,w] = depth[b,n,h,w] * (c0*w + c1*h + c2) + c3,  c* = p2p[b,n,i,:]

Data-parallel over the 24 (b,n) pairs: 3 pairs per core on 8 cores.
Memory-bound; all HBM traffic is bf16 (host converts): 2.95 MiB depth in
+ 8.85 MiB out per core (vs 23.6 MiB for f32), rel-err ~4e-3 << 2e-2.

Per-core compute, per (pair, t) row-block tile [128, 960] (h = p + 128t):
  UD  = u * d                （tensor_tensor, shared by the 3 channels)
  q_i = d * rows_i + c3_i    (rows_i[p] = c1*(p+128t)+c2; ACT or tensor_scalar)
  o_i = UD * c0_i + q_i      (native scalar_tensor_tensor)
All ops native (no slow custom-DVE ucode), spread across vector/scalar/
gpsimd so no engine exceeds the ~30 us DMA roofline.  Stores ride the
Sync + GpSimd HWDGE rings; depth loads ride the Scalar ring.
"""

from contextlib import ExitStack

import numpy as np
import ml_dtypes

import concourse.bacc as bacc
import concourse.mybir as mybir
import concourse.tile as tile
from concourse.bass_utils import run_bass_kernel_spmd

F32 = mybir.dt.float32
BF16 = mybir.dt.bfloat16
I32 = mybir.dt.int32
NP_BF16 = ml_dtypes.bfloat16

B, N, H, W = 4, 6, 512, 960
NCORES = 8
PAIRS = B * N           # 24
PPC = PAIRS // NCORES   # 3 (b,n) pairs per core
PB = 128                # SBUF partitions
NB = H // PB            # 4 row blocks per image

_cached_nc = None


def _build_bass():
    nc = bacc.Bacc("TRN2", target_bir_lowering=False, debug=False)
    depth = nc.dram_tensor("depth", [PPC * H, W], BF16, kind="ExternalInput")
    p2p = nc.dram_tensor("p2p", [PB, PPC * 16], F32, kind="ExternalInput")
    # out row = pair*H + h ; col = i*W + w  (host untangles channels)
    out = nc.dram_tensor("out", [PPC * H, 3 * W], BF16, kind="ExternalOutput")

    mult = mybir.AluOpType.mult
    add = mybir.AluOpType.add
    ident = mybir.ActivationFunctionType.Identity

    with tile.TileContext(nc) as tc, ExitStack() as ctx:
        const = ctx.enter_context(tc.tile_pool(name="const", bufs=1))
        dpool = ctx.enter_context(tc.tile_pool(name="dp", bufs=1))
        udpool = ctx.enter_context(tc.tile_pool(name="ud", bufs=3))
        qpool = ctx.enter_context(tc.tile_pool(name="qp", bufs=6))
        opool = ctx.enter_context(tc.tile_pool(name="op", bufs=4))

        # tiny coef load first so it never queues behind bulk traffic
        coef = const.tile([PB, PPC * 16], F32)
        nc.sync.dma_start(coef[:], p2p[:])

        # index grids: u[p,w] = w (bf16), v[p,t] = p + 128t (f32)
        u_i32 = const.tile([PB, W], I32)
        nc.gpsimd.iota(u_i32[:], [[1, W]], base=0, channel_multiplier=0)
        u_bf = const.tile([PB, W], BF16)
        nc.vector.tensor_copy(u_bf[:], u_i32[:])
        v_i32 = const.tile([PB, NB], I32)
        nc.gpsimd.iota(v_i32[:], [[PB, NB]], base=0, channel_multiplier=1)
        v_sb = const.tile([PB, NB], F32)
        nc.vector.tensor_copy(v_sb[:], v_i32[:])

        # whole-pair depth loads: partition p, block t <- DRAM row t*128+p
        d_tiles = []
        for pair in range(PPC):
            d = dpool.tile([PB, NB, W], BF16, tag=f"d{pair}")
            dview = depth[pair * H:(pair + 1) * H, :].rearrange(
                "(t p) w -> p t w", p=PB)
            nc.scalar.dma_start(d[:], dview)
            d_tiles.append(d)

        # rows[p, (pair*3+i)*NB + t] = c1*(p + 128t) + c2   (f32 scalars)
        rows = const.tile([PB, PPC * 3 * NB], F32)
        for pair in range(PPC):
            for i in range(3):
                g = (pair * 3 + i) * NB
                cb = 16 * pair + 4 * i
                nc.vector.tensor_scalar(
                    rows[:, g:g + NB], v_sb[:],
                    coef[:, cb + 1:cb + 2], coef[:, cb + 2:cb + 3],
                    mult, add)

        store_rings = [nc.sync, nc.gpsimd]
        for pair in range(PPC):
            d = d_tiles[pair]
            for t in range(NB):
                d_t = d[:, t, :]
                ud = udpool.tile([PB, W], BF16)
                nc.gpsimd.tensor_tensor(ud[:], u_bf[:], d_t, mult)
                o4 = opool.tile([PB, 3, W], BF16)
                for i in range(3):
                    cb = 16 * pair + 4 * i
                    g = (pair * 3 + i) * NB
                    q = qpool.tile([PB, W], BF16)
                    if i == 0:
                        # vector takes one q per tile-triple
                        nc.vector.tensor_scalar(
                            q[:], d_t,
                            rows[:, g + t:g + t + 1],
                            coef[:, cb + 3:cb + 4],
                            mult, add)
                    else:
                        nc.scalar.activation(
                            q[:], d_t, ident,
                            bias=coef[:, cb + 3:cb + 4],
                            scale=rows[:, g + t:g + t + 1])
                    nc.vector.scalar_tensor_tensor(
                        o4[:, i, :], ud[:], coef[:, cb:cb + 1], q[:],
                        mult, add)
                ov = out[pair * H + t * PB: pair * H + (t + 1) * PB, :]
                ring = store_rings[(pair * NB + t) % 2]
                ring.dma_start(ov.rearrange("p (i w) -> p i w", i=3), o4[:])
    nc.compile()
    return nc


def _make_in_maps(depth, p2p):
    dflat = np.ascontiguousarray(
        np.asarray(depth, dtype=np.float32)).reshape(PAIRS, H, W)
    pflat = np.ascontiguousarray(
        np.asarray(p2p, dtype=np.float32)).reshape(PAIRS, 16)
    in_maps = []
    for c in range(NCORES):
        sl = slice(c * PPC, (c + 1) * PPC)
        in_maps.append({
            "depth": np.ascontiguousarray(
                dflat[sl].reshape(PPC * H, W).astype(NP_BF16)),
            "p2p": np.ascontiguousarray(np.broadcast_to(
                pflat[sl].reshape(1, PPC * 16), (PB, PPC * 16))),
        })
    return in_maps


def _gather(results):
    outs = [
        np.asarray(r["out"]).reshape(PPC, H, 3, W).transpose(0, 2, 1, 3)
        for r in results
    ]
    return np.concatenate(outs, axis=0).astype(np.float32).reshape(
        B, N, 3, H, W)


def kernel(depth, p2p):
    global _cached_nc
    if _cached_nc is None:
        _cached_nc = _build_bass()
    in_maps = _make_in_maps(depth, p2p)
    res = run_bass_kernel_spmd(_cached_nc, in_maps, list(range(NCORES)))
    return _gather(res.results)
